# revision 14
# baseline (speedup 1.0000x reference)
"""GQA attention (B=2, LQ=LK=2048, D=2048, H=16, KV=4, dh=128) on 8 TRN2 cores.

Sharding: core = b*4 + kv  (data parallel over batch, tensor parallel over
kv-head groups). Each core projects Q (its 4 heads) / K / V (its kv head),
runs attention with position bias, and computes its column-shard of the
output projection; the 4 partial outputs per batch are summed on host.

All matmuls run as float32r (TF32-like fp32 path, 1 cycle/row at N>=256).
Layouts are chosen so no on-device transposes are needed:
  - activations enter as hq^T / hkv^T  [dm, l]
  - S is computed transposed: S^T[lk, lq] = (K^T)^T-chunk contraction
  - softmax denominator via ones-vector matmul (partition-dim reduction on PE)
  - O^T[dh, lq] accumulates directly with V chunks as stationary operand
  - output projection emits out^T[dm, lq]; host sums partials + transposes
"""

import numpy as np
import ml_dtypes

import concourse.bass as bass
import concourse.tile as tile
from concourse import bacc, mybir
from concourse.bass_utils import run_bass_kernel_spmd

DM = 2048      # model dim
LQ = 2048
LK = 2048
DH = 128       # head dim
H = 16         # query heads
KV = 4         # kv heads
G = H // KV    # query heads per kv head (4)
B = 2
KC = DM // 128   # contraction chunks (16)
LKC = LK // 128  # lk chunks (16)
NQT = 4          # lq tiles of 512
LQT = LQ // NQT  # 512

f32 = mybir.dt.float32
f32r = mybir.dt.float32r
bf16 = mybir.dt.bfloat16

_BUILT = None


def _build():
    nc = bacc.Bacc()
    hqT = nc.declare_dram_parameter("hqT", [DM, LQ], bf16, isOutput=False)
    hkvT = nc.declare_dram_parameter("hkvT", [DM, LK], bf16, isOutput=False)
    # weights pre-reshaped on host to SBUF layout [128, ...] (see kernel())
    wq = nc.declare_dram_parameter("wq", [128, KC * G * DH], bf16, isOutput=False)
    wk = nc.declare_dram_parameter("wk", [128, KC * DH], bf16, isOutput=False)
    wv = nc.declare_dram_parameter("wv", [128, KC * DH], bf16, isOutput=False)
    ident = nc.declare_dram_parameter("ident", [128, 128], bf16, isOutput=False)
    wo = nc.declare_dram_parameter("wo", [128, G * DM], f32r, isOutput=False)
    biasT = nc.declare_dram_parameter("biasT", [G, LK, LQ], bf16, isOutput=False)
    ones_in = nc.declare_dram_parameter("ones_in", [128, 1], f32r, isOutput=False)
    ones_row = nc.declare_dram_parameter("ones_row", [1, 128], f32r, isOutput=False)
    outT = nc.declare_dram_parameter("outT", [DM, LQ], f32, isOutput=True)

    GW = G * DH  # 512, per-core q-head width

    with tile.TileContext(nc) as tc:
        with (
            tc.tile_pool(name="persist", bufs=1) as pp,
        ):
            ones = pp.tile([128, 1], f32r)
            nc.sync.dma_start(ones[:], ones_in[:])
            ident_sb = pp.tile([128, 128], bf16)
            nc.sync.dma_start(ident_sb[:], ident[:])
            ones_b = pp.tile([128, 1], bf16)
            nc.vector.memset(ones_b[:], 1.0)
            ones_r1 = pp.tile([1, 128], f32r)
            nc.sync.dma_start(ones_r1[:], ones_row[:])

            kt_sb = pp.tile([128, LK], bf16)          # K^T [dh, lk]
            v_sb = pp.tile([128, LKC * DH], bf16)     # V chunks [lk%128, c*dh]
            qt_sb = pp.tile([128, G * LQ], bf16)      # Q^T per head 2MB
            ot_sb = pp.tile([128, G * LQ], f32r)      # O^T per head 4MB

            wop = tc.alloc_tile_pool(name="wob", bufs=1)
            wo_sb = wop.tile([128, G * DM], f32r)  # 4MB, needed in phase D
            nc.sync.dma_start(wo_sb[:], wo[:])
            wp = tc.alloc_tile_pool(name="wqkv", bufs=1)
            wq_sb = wp.tile([128, KC * GW], bf16)     # 2MB
            nc.sync.dma_start(wq_sb[:], wq[:])
            wk_sb = wp.tile([128, KC * DH], bf16)
            nc.sync.dma_start(wk_sb[:], wk[:])
            wv_sb = wp.tile([128, KC * DH], bf16)
            nc.sync.dma_start(wv_sb[:], wv[:])

            # ---- Phase A: K^T and V from hkvT ----
            with (
                tc.tile_pool(name="slabs", bufs=5) as slabp,
                tc.tile_pool(name="ps_a", bufs=1, space="PSUM") as psa,
            ):
                ps_kt = psa.tile([128, LK], f32)      # 4 banks
                ps_v = psa.tile([128, LKC * DH], f32)  # 4 banks
                for kc in range(KC):
                    slab = slabp.tile([128, LK], bf16)
                    nc.sync.dma_start(slab[:], hkvT[kc * 128:(kc + 1) * 128, :])
                    for n in range(LK // 512):
                        nc.tensor.matmul(
                            ps_kt[:, n * 512:(n + 1) * 512],
                            wk_sb[:, kc * DH:(kc + 1) * DH],
                            slab[:, n * 512:(n + 1) * 512],
                            start=(kc == 0), stop=(kc == KC - 1),
                        )
                    for m in range(LKC):
                        # start=True clears has_written for the WHOLE PSUM
                        # bank, so only the first write into each bank (4
                        # m-tiles of 128 cols share a 512-col bank) may set it.
                        nc.tensor.matmul(
                            ps_v[:, m * DH:(m + 1) * DH],
                            slab[:, m * 128:(m + 1) * 128],
                            wv_sb[:, kc * DH:(kc + 1) * DH],
                            start=(kc == 0 and m % 4 == 0), stop=(kc == KC - 1),
                            skip_group_check=True,
                        )
                nc.vector.tensor_copy(kt_sb[:], ps_kt[:])
                nc.vector.tensor_copy(v_sb[:], ps_v[:])

            # ---- Phase B: Q^T (4 heads) from hqT ----
            with (
                tc.tile_pool(name="slabs_b", bufs=6) as slabp,
                tc.tile_pool(name="ps_b", bufs=1, space="PSUM") as psb,
            ):
                for lqh in range(2):  # lq halves of 1024
                    ps_q = psb.tile([128, G * 1024], f32)  # 8 banks
                    for kc in range(KC):
                        slab = slabp.tile([128, 1024], bf16)
                        nc.sync.dma_start(
                            slab[:],
                            hqT[kc * 128:(kc + 1) * 128, lqh * 1024:(lqh + 1) * 1024],
                        )
                        for h in range(G):
                            for n in range(2):
                                nc.tensor.matmul(
                                    ps_q[:, h * 1024 + n * 512: h * 1024 + (n + 1) * 512],
                                    wq_sb[:, kc * GW + h * DH: kc * GW + (h + 1) * DH],
                                    slab[:, n * 512:(n + 1) * 512],
                                    start=(kc == 0), stop=(kc == KC - 1),
                                )
                    for h in range(G):
                        nc.vector.tensor_copy(
                            qt_sb[:, h * LQ + lqh * 1024: h * LQ + (lqh + 1) * 1024],
                            ps_q[:, h * 1024:(h + 1) * 1024],
                        )

            wp.release()

            # ---- Phase C: attention per (head, lq tile) ----
            # Per lk-chunk: PE writes bias into PSUM (identity matmul,
            # start=True sets has_written), S-matmul accumulates scores on
            # top, ACT does exp, PE accumulates O^T; DVE accumulates the
            # softmax denominator (partition tree + tiny K=32 matmul at tile
            # end). Normalization is deferred to a dense post-pass (C2).
            with (
                tc.tile_pool(name="biasb", bufs=6) as biasp,
                tc.tile_pool(name="ptb", bufs=4) as ptp,
                tc.tile_pool(name="accb", bufs=2) as accp,
                tc.tile_pool(name="smallb", bufs=3) as smallp,
                tc.tile_pool(name="rsb", bufs=1) as rsp,
                tc.tile_pool(name="ps_s", bufs=4, space="PSUM") as pss,
                tc.tile_pool(name="ps_o", bufs=2, space="PSUM") as pso,
                tc.tile_pool(name="ps_r", bufs=1, space="PSUM") as psr,
                tc.tile_pool(name="ps_bc", bufs=1, space="PSUM") as psbc,
            ):
                rs_sb = rsp.tile([1, G * NQT * LQT], f32)   # per-tile raw rowsums
                rs8 = [rsp.tile([8, LQT], f32, name=f"rs8_{i}") for i in range(2)]
                rc8 = [rsp.tile([8, LQT], f32, name=f"rc8_{i}") for i in range(2)]
                rc8r = [rsp.tile([8, LQT], f32r, name=f"rc8r_{i}") for i in range(2)]
                rs_r = rsp.tile([1, G * NQT * LQT], f32r)   # 1/rowsum, row layout

                def s_chunk(h, t, c):
                    """exp(bias + S^T) for lk-chunk c of tile (h, t)."""
                    q_off = h * LQ + t * LQT
                    ps_s = pss.tile([128, LQT], f32)
                    bt = biasp.tile([128, LQT], bf16)
                    nc.sync.dma_start(
                        bt[:],
                        biasT[h, c * 128:(c + 1) * 128, t * LQT:(t + 1) * LQT],
                    )
                    nc.tensor.matmul(ps_s[:], ident_sb[:], bt[:], start=True, stop=False)
                    nc.tensor.matmul(
                        ps_s[:],
                        kt_sb[:, c * 128:(c + 1) * 128],
                        qt_sb[:, q_off:q_off + LQT],
                        start=False, stop=True,
                        skip_group_check=True,
                    )
                    pt = ptp.tile([128, LQT], bf16)
                    nc.scalar.activation(
                        pt[:], ps_s[:], mybir.ActivationFunctionType.Exp
                    )
                    return pt

                tiles = [(h, t) for h in range(G) for t in range(NQT)]
                state = {}
                LOOKAHEAD = 3
                flat = [(h, t, c) for h, t in tiles for c in range(LKC)]
                for i in range(LOOKAHEAD):
                    pt_pre = s_chunk(*flat[i])
                    state[flat[i]] = pt_pre
                for i, (h, t, c) in enumerate(flat):
                    if i + LOOKAHEAD < len(flat):
                        state[flat[i + LOOKAHEAD]] = s_chunk(*flat[i + LOOKAHEAD])
                    pt = state.pop((h, t, c))
                    q_off = h * LQ + t * LQT
                    if c == 0:
                        state[("o", h, t)] = pso.tile([128, LQT], f32, name="ps_o", tag="ps_o")
                        state[("a", h, t)] = accp.tile([128, LQT], bf16, name="acc", tag="acc")
                    ps_o = state[("o", h, t)]
                    acc = state[("a", h, t)]
                    nc.tensor.matmul(
                        ps_o[:],
                        v_sb[:, c * DH:(c + 1) * DH],
                        pt[:],
                        start=(c == 0), stop=(c == LKC - 1),
                    )
                    if c == 0:
                        nc.vector.tensor_copy(acc[:], pt[:])
                    else:
                        nc.vector.tensor_tensor(
                            acc[:], acc[:], pt[:], op=mybir.AluOpType.add
                        )
                    if c == LKC - 1:
                        ps_o = state.pop(("o", h, t))
                        acc = state.pop(("a", h, t))
                        idx = h * NQT + t
                        slot = idx * LQT
                        ps_r = psr.tile([1, LQT], f32)
                        nc.tensor.matmul(
                            ps_r[:], ones_b[:], acc[:], start=True, stop=True
                        )
                        nc.vector.tensor_copy(rs_sb[:, slot:slot + LQT], ps_r[:])
                        nc.sync.dma_start(
                            rs8[idx // 8][idx % 8:idx % 8 + 1, :],
                            rs_sb[:, slot:slot + LQT],
                        )
                        # unnormalized O^T eviction (normalized in C2)
                        nc.vector.tensor_copy(
                            ot_sb[:, q_off:q_off + LQT], ps_o[:]
                        )
                        if idx % 8 == 7:
                            # batch-reciprocal this half; C2 for its 8 tiles
                            bb = idx // 8
                            nc.vector.reciprocal(rc8[bb][:], rs8[bb][:])
                            nc.scalar.activation(
                                rc8r[bb][:], rc8[bb][:],
                                mybir.ActivationFunctionType.Copy,
                            )
                            for j in range(8):
                                k = bb * 8 + j
                                nc.sync.dma_start(
                                    rs_r[:, k * LQT:(k + 1) * LQT],
                                    rc8r[bb][j:j + 1, :],
                                )
                            for k in range(bb * 8, bb * 8 + 8):
                                hh, tt = divmod(k, NQT)
                                qo = hh * LQ + tt * LQT
                                ps_bc = psbc.tile([128, LQT], f32, name="ps_bc")
                                nc.tensor.matmul(
                                    ps_bc[:], ones_r1[:],
                                    rs_r[:, k * LQT:(k + 1) * LQT],
                                    start=True, stop=True,
                                )
                                nc.vector.tensor_tensor(
                                    ot_sb[:, qo:qo + LQT],
                                    ot_sb[:, qo:qo + LQT], ps_bc[:],
                                    op=mybir.AluOpType.mult,
                                )

            # ---- Phase D: output projection (column shard) ----
            with (
                tc.tile_pool(name="outb", bufs=3) as outp,
                tc.tile_pool(name="ps_d", bufs=2, space="PSUM") as psd,
            ):
                for dmt in range(DM // 128):
                    ps_out = psd.tile([128, LQ], f32)  # 4 banks
                    for h in range(G):
                        for n in range(LQ // 512):
                            nc.tensor.matmul(
                                ps_out[:, n * 512:(n + 1) * 512],
                                wo_sb[:, h * DM + dmt * 128: h * DM + (dmt + 1) * 128],
                                ot_sb[:, h * LQ + n * 512: h * LQ + (n + 1) * 512],
                                start=(h == 0), stop=(h == G - 1),
                            )
                    o_out = outp.tile([128, LQ], f32)
                    if dmt % 2 == 0:
                        nc.vector.tensor_copy(o_out[:], ps_out[:])
                    else:
                        nc.scalar.activation(
                            o_out[:], ps_out[:], mybir.ActivationFunctionType.Copy
                        )
                    nc.sync.dma_start(outT[dmt * 128:(dmt + 1) * 128, :], o_out[:])
            wop.release()

    nc.finalize()
    return nc


def _get_nc():
    global _BUILT
    if _BUILT is None:
        _BUILT = _build()
    return _BUILT


def kernel(hidden_q, hidden_kv, attention_mask, position_bias, Wq, Wk, Wv, Wo,
           _trace=False):
    hidden_q = np.asarray(hidden_q, np.float32)
    hidden_kv = np.asarray(hidden_kv, np.float32)
    position_bias = np.asarray(position_bias, np.float32)
    Wq = np.asarray(Wq, np.float32)
    Wk = np.asarray(Wk, np.float32)
    Wv = np.asarray(Wv, np.float32)
    Wo = np.asarray(Wo, np.float32)
    # attention_mask is all-ones by problem spec; masking is a no-op.

    inv4 = 1.0 / np.sqrt(np.sqrt(np.float32(DH)))
    GW = G * DH

    def sb_layout(a, cast_bf16=True):
        # [dm, w] -> [128, KC*w] with contraction chunk kc at cols [kc*w,(kc+1)*w)
        w = a.shape[1]
        out = np.ascontiguousarray(
            a.reshape(KC, 128, w).transpose(1, 0, 2).reshape(128, KC * w)
        )
        return np.asarray(out.astype(ml_dtypes.bfloat16)) if cast_bf16 else out

    # per-kv weight shards
    wq_s, wk_s, wv_s, wo_s, biasT_s = [], [], [], [], []
    WqT = (Wq.T * inv4).astype(np.float32)   # [dm, H*dh]
    WkT = (Wk.T * inv4).astype(np.float32)   # [dm, KV*dh]
    WvT = Wv.T.astype(np.float32)            # [dm, KV*dh]
    for kv in range(KV):
        wq_s.append(sb_layout(np.ascontiguousarray(WqT[:, kv * GW:(kv + 1) * GW])))
        wk_s.append(sb_layout(np.ascontiguousarray(WkT[:, kv * DH:(kv + 1) * DH])))
        wv_s.append(sb_layout(np.ascontiguousarray(WvT[:, kv * DH:(kv + 1) * DH])))
        # wo layout: [128(dh), G*dm]; head h cols = Wo[:, kv*GW+h*DH : +DH].T
        wo_kv = Wo[:, kv * GW:(kv + 1) * GW].T  # [GW, dm]
        wo_s.append(np.ascontiguousarray(
            wo_kv.reshape(G, DH, DM).transpose(1, 0, 2).reshape(128, G * DM)
        ))  # stays f32 (f32r matmul in phase D)
        biasT_s.append(np.ascontiguousarray(
            position_bias[kv * G:(kv + 1) * G].transpose(0, 2, 1)
        ).astype(ml_dtypes.bfloat16))

    hqT = [np.asarray(np.ascontiguousarray(hidden_q[b].T).astype(ml_dtypes.bfloat16))
           for b in range(B)]
    hkvT = [np.asarray(np.ascontiguousarray(hidden_kv[b].T).astype(ml_dtypes.bfloat16))
            for b in range(B)]
    ones_arr = np.ones((128, 1), np.float32)
    ident_arr = np.asarray(np.eye(128, dtype=np.float32).astype(ml_dtypes.bfloat16))
    ones_row_arr = np.ones((1, 128), np.float32)

    in_maps = []
    for core in range(8):
        b, kv = divmod(core, KV)
        in_maps.append({
            "hqT": hqT[b], "hkvT": hkvT[b],
            "wq": wq_s[kv], "wk": wk_s[kv], "wv": wv_s[kv], "wo": wo_s[kv],
            "biasT": np.asarray(biasT_s[kv]),
            "ones_in": ones_arr, "ones_row": ones_row_arr, "ident": ident_arr,
        })

    nc = _get_nc()
    res = run_bass_kernel_spmd(nc, in_maps, core_ids=list(range(8)), trace=_trace)
    kernel.last_exec_time_ns = res.exec_time_ns

    out = np.empty((B, LQ, DM), np.float32)
    for b in range(B):
        acc = res.results[b * KV]["outT"].astype(np.float64)
        for kv in range(1, KV):
            acc += res.results[b * KV + kv]["outT"]
        out[b] = acc.T.astype(np.float32)
    return out


# revision 15
# speedup vs baseline: 1.0666x; 1.0666x over previous
"""GQA attention (B=2, LQ=LK=2048, D=2048, H=16, KV=4, dh=128) on 8 TRN2 cores.

Sharding: core = b*4 + kv  (data parallel over batch, tensor parallel over
kv-head groups). Each core projects Q (its 4 heads) / K / V (its kv head),
runs attention with position bias, and computes its column-shard of the
output projection; the 4 partial outputs per batch are summed on host.

All matmuls run as float32r (TF32-like fp32 path, 1 cycle/row at N>=256).
Layouts are chosen so no on-device transposes are needed:
  - activations enter as hq^T / hkv^T  [dm, l]
  - S is computed transposed: S^T[lk, lq] = (K^T)^T-chunk contraction
  - softmax denominator via ones-vector matmul (partition-dim reduction on PE)
  - O^T[dh, lq] accumulates directly with V chunks as stationary operand
  - output projection emits out^T[dm, lq]; host sums partials + transposes
"""

import numpy as np
import ml_dtypes

import concourse.bass as bass
import concourse.tile as tile
from concourse import bacc, mybir
from concourse.bass_utils import run_bass_kernel_spmd

DM = 2048      # model dim
LQ = 2048
LK = 2048
DH = 128       # head dim
H = 16         # query heads
KV = 4         # kv heads
G = H // KV    # query heads per kv head (4)
B = 2
KC = DM // 128   # contraction chunks (16)
LKC = LK // 128  # lk chunks (16)
NQT = 4          # lq tiles of 512
LQT = LQ // NQT  # 512

f32 = mybir.dt.float32
f32r = mybir.dt.float32r
bf16 = mybir.dt.bfloat16

_BUILT = None


def _build():
    nc = bacc.Bacc()
    hqT = nc.declare_dram_parameter("hqT", [DM, LQ], bf16, isOutput=False)
    hkvT = nc.declare_dram_parameter("hkvT", [DM, LK], bf16, isOutput=False)
    # weights pre-reshaped on host to SBUF layout [128, ...] (see kernel())
    wq = nc.declare_dram_parameter("wq", [128, KC * G * DH], bf16, isOutput=False)
    wk = nc.declare_dram_parameter("wk", [128, KC * DH], bf16, isOutput=False)
    wv = nc.declare_dram_parameter("wv", [128, KC * DH], bf16, isOutput=False)
    ident = nc.declare_dram_parameter("ident", [128, 128], bf16, isOutput=False)
    wo = nc.declare_dram_parameter("wo", [128, G * DM], bf16, isOutput=False)
    biasT = nc.declare_dram_parameter("biasT", [G, LK, LQ], bf16, isOutput=False)
    ones_in = nc.declare_dram_parameter("ones_in", [128, 1], f32r, isOutput=False)
    ones_row = nc.declare_dram_parameter("ones_row", [1, 128], f32r, isOutput=False)
    outT = nc.declare_dram_parameter("outT", [DM, LQ], f32, isOutput=True)

    GW = G * DH  # 512, per-core q-head width

    with tile.TileContext(nc) as tc:
        with (
            tc.tile_pool(name="persist", bufs=1) as pp,
        ):
            ones = pp.tile([128, 1], f32r)
            nc.sync.dma_start(ones[:], ones_in[:])
            ident_sb = pp.tile([128, 128], bf16)
            nc.sync.dma_start(ident_sb[:], ident[:])
            ones_b = pp.tile([128, 1], bf16)
            nc.vector.memset(ones_b[:], 1.0)
            ones_r1 = pp.tile([1, 128], f32r)
            nc.sync.dma_start(ones_r1[:], ones_row[:])

            kt_sb = pp.tile([128, LK], bf16)          # K^T [dh, lk]
            v_sb = pp.tile([128, LKC * DH], bf16)     # V chunks [lk%128, c*dh]
            qt_sb = pp.tile([128, G * LQ], bf16)      # Q^T per head 2MB
            ot_sb = pp.tile([128, G * LQ], bf16)      # O^T per head 2MB

            wop = tc.alloc_tile_pool(name="wob", bufs=1)
            wo_sb = wop.tile([128, G * DM], bf16)  # 2MB, needed in phase D
            nc.sync.dma_start(wo_sb[:], wo[:])
            wp = tc.alloc_tile_pool(name="wqkv", bufs=1)
            wq_sb = wp.tile([128, KC * GW], bf16)     # 2MB
            nc.sync.dma_start(wq_sb[:], wq[:])
            wk_sb = wp.tile([128, KC * DH], bf16)
            nc.sync.dma_start(wk_sb[:], wk[:])
            wv_sb = wp.tile([128, KC * DH], bf16)
            nc.sync.dma_start(wv_sb[:], wv[:])

            # ---- Phase A: K^T and V from hkvT ----
            with (
                tc.tile_pool(name="slabs", bufs=5) as slabp,
                tc.tile_pool(name="ps_a", bufs=1, space="PSUM") as psa,
            ):
                ps_kt = psa.tile([128, LK], f32)      # 4 banks
                ps_v = psa.tile([128, LKC * DH], f32)  # 4 banks
                for kc in range(KC):
                    slab = slabp.tile([128, LK], bf16)
                    nc.sync.dma_start(slab[:], hkvT[kc * 128:(kc + 1) * 128, :])
                    for n in range(LK // 512):
                        nc.tensor.matmul(
                            ps_kt[:, n * 512:(n + 1) * 512],
                            wk_sb[:, kc * DH:(kc + 1) * DH],
                            slab[:, n * 512:(n + 1) * 512],
                            start=(kc == 0), stop=(kc == KC - 1),
                        )
                    for m in range(LKC):
                        # start=True clears has_written for the WHOLE PSUM
                        # bank, so only the first write into each bank (4
                        # m-tiles of 128 cols share a 512-col bank) may set it.
                        nc.tensor.matmul(
                            ps_v[:, m * DH:(m + 1) * DH],
                            slab[:, m * 128:(m + 1) * 128],
                            wv_sb[:, kc * DH:(kc + 1) * DH],
                            start=(kc == 0 and m % 4 == 0), stop=(kc == KC - 1),
                            skip_group_check=True,
                        )
                nc.vector.tensor_copy(kt_sb[:], ps_kt[:])
                nc.vector.tensor_copy(v_sb[:], ps_v[:])

            # ---- Phase B: Q^T (4 heads) from hqT ----
            with (
                tc.tile_pool(name="slabs_b", bufs=6) as slabp,
                tc.tile_pool(name="ps_b", bufs=1, space="PSUM") as psb,
            ):
                for lqh in range(2):  # lq halves of 1024
                    ps_q = psb.tile([128, G * 1024], f32)  # 8 banks
                    for kc in range(KC):
                        slab = slabp.tile([128, 1024], bf16)
                        nc.sync.dma_start(
                            slab[:],
                            hqT[kc * 128:(kc + 1) * 128, lqh * 1024:(lqh + 1) * 1024],
                        )
                        for h in range(G):
                            for n in range(2):
                                nc.tensor.matmul(
                                    ps_q[:, h * 1024 + n * 512: h * 1024 + (n + 1) * 512],
                                    wq_sb[:, kc * GW + h * DH: kc * GW + (h + 1) * DH],
                                    slab[:, n * 512:(n + 1) * 512],
                                    start=(kc == 0), stop=(kc == KC - 1),
                                )
                    for h in range(G):
                        nc.vector.tensor_copy(
                            qt_sb[:, h * LQ + lqh * 1024: h * LQ + (lqh + 1) * 1024],
                            ps_q[:, h * 1024:(h + 1) * 1024],
                        )

            wp.release()

            # ---- Phase C: attention per (head, lq tile) ----
            # Per lk-chunk: PE writes bias into PSUM (identity matmul,
            # start=True sets has_written), S-matmul accumulates scores on
            # top, ACT does exp, PE accumulates O^T; DVE accumulates the
            # softmax denominator (partition tree + tiny K=32 matmul at tile
            # end). Normalization is deferred to a dense post-pass (C2).
            with (
                tc.tile_pool(name="biasb", bufs=6) as biasp,
                tc.tile_pool(name="ptb", bufs=4) as ptp,
                tc.tile_pool(name="accb", bufs=2) as accp,
                tc.tile_pool(name="smallb", bufs=3) as smallp,
                tc.tile_pool(name="rsb", bufs=1) as rsp,
                tc.tile_pool(name="ps_s", bufs=3, space="PSUM") as pss,
                tc.tile_pool(name="ps_o", bufs=2, space="PSUM") as pso,
                tc.tile_pool(name="ps_r", bufs=1, space="PSUM") as psr,
                tc.tile_pool(name="ps_bc", bufs=2, space="PSUM") as psbc,
            ):
                rs_sb = rsp.tile([1, G * NQT * LQT], f32)   # per-tile raw rowsums
                rs8 = [rsp.tile([8, LQT], f32, name=f"rs8_{i}") for i in range(2)]
                rc8 = [rsp.tile([8, LQT], f32, name=f"rc8_{i}") for i in range(2)]
                rc8r = [rsp.tile([8, LQT], f32r, name=f"rc8r_{i}") for i in range(2)]
                rs_r = rsp.tile([1, G * NQT * LQT], f32r)   # 1/rowsum, row layout

                def s_chunk(h, t, c):
                    """exp(bias + S^T) for lk-chunk c of tile (h, t)."""
                    q_off = h * LQ + t * LQT
                    ps_s = pss.tile([128, LQT], f32)
                    bt = biasp.tile([128, LQT], bf16)
                    nc.sync.dma_start(
                        bt[:],
                        biasT[h, c * 128:(c + 1) * 128, t * LQT:(t + 1) * LQT],
                    )
                    nc.tensor.matmul(ps_s[:], ident_sb[:], bt[:], start=True, stop=False)
                    nc.tensor.matmul(
                        ps_s[:],
                        kt_sb[:, c * 128:(c + 1) * 128],
                        qt_sb[:, q_off:q_off + LQT],
                        start=False, stop=True,
                        skip_group_check=True,
                    )
                    pt = ptp.tile([128, LQT], bf16)
                    nc.scalar.activation(
                        pt[:], ps_s[:], mybir.ActivationFunctionType.Exp
                    )
                    return pt

                tiles = [(h, t) for h in range(G) for t in range(NQT)]
                state = {}
                LOOKAHEAD = 2
                flat = [(h, t, c) for h, t in tiles for c in range(LKC)]
                for i in range(LOOKAHEAD):
                    pt_pre = s_chunk(*flat[i])
                    state[flat[i]] = pt_pre
                for i, (h, t, c) in enumerate(flat):
                    if i + LOOKAHEAD < len(flat):
                        state[flat[i + LOOKAHEAD]] = s_chunk(*flat[i + LOOKAHEAD])
                    pt = state.pop((h, t, c))
                    q_off = h * LQ + t * LQT
                    if c == 0:
                        state[("o", h, t)] = pso.tile([128, LQT], f32, name="ps_o", tag="ps_o")
                        state[("a", h, t)] = accp.tile([128, LQT], bf16, name="acc", tag="acc")
                    ps_o = state[("o", h, t)]
                    acc = state[("a", h, t)]
                    nc.tensor.matmul(
                        ps_o[:],
                        v_sb[:, c * DH:(c + 1) * DH],
                        pt[:],
                        start=(c == 0), stop=(c == LKC - 1),
                    )
                    if c == 0:
                        nc.vector.tensor_copy(acc[:], pt[:])
                    else:
                        nc.vector.tensor_tensor(
                            acc[:], acc[:], pt[:], op=mybir.AluOpType.add
                        )
                    if c == LKC - 1:
                        ps_o = state.pop(("o", h, t))
                        acc = state.pop(("a", h, t))
                        idx = h * NQT + t
                        slot = idx * LQT
                        ps_r = psr.tile([1, LQT], f32)
                        nc.tensor.matmul(
                            ps_r[:], ones_b[:], acc[:], start=True, stop=True
                        )
                        nc.vector.tensor_copy(rs_sb[:, slot:slot + LQT], ps_r[:])
                        nc.sync.dma_start(
                            rs8[idx // 8][idx % 8:idx % 8 + 1, :],
                            rs_sb[:, slot:slot + LQT],
                        )
                        # unnormalized O^T eviction (normalized in C2)
                        nc.vector.tensor_copy(
                            ot_sb[:, q_off:q_off + LQT], ps_o[:]
                        )
                        if idx % 8 == 7:
                            # batch-reciprocal this half; C2 for its 8 tiles
                            bb = idx // 8
                            nc.vector.reciprocal(rc8[bb][:], rs8[bb][:])
                            nc.scalar.activation(
                                rc8r[bb][:], rc8[bb][:],
                                mybir.ActivationFunctionType.Copy,
                            )
                            for j in range(8):
                                k = bb * 8 + j
                                nc.sync.dma_start(
                                    rs_r[:, k * LQT:(k + 1) * LQT],
                                    rc8r[bb][j:j + 1, :],
                                )
                            for k in range(bb * 8, bb * 8 + 8):
                                hh, tt = divmod(k, NQT)
                                qo = hh * LQ + tt * LQT
                                ps_bc = psbc.tile([128, LQT], f32, name="ps_bc")
                                nc.tensor.matmul(
                                    ps_bc[:], ones_r1[:],
                                    rs_r[:, k * LQT:(k + 1) * LQT],
                                    start=True, stop=True,
                                )
                                nc.vector.tensor_tensor(
                                    ot_sb[:, qo:qo + LQT],
                                    ot_sb[:, qo:qo + LQT], ps_bc[:],
                                    op=mybir.AluOpType.mult,
                                )

            # ---- Phase D: output projection (column shard) ----
            with (
                tc.tile_pool(name="outb", bufs=3) as outp,
                tc.tile_pool(name="ps_d", bufs=2, space="PSUM") as psd,
            ):
                for dmt in range(DM // 128):
                    ps_out = psd.tile([128, LQ], f32)  # 4 banks
                    for h in range(G):
                        for n in range(LQ // 512):
                            nc.tensor.matmul(
                                ps_out[:, n * 512:(n + 1) * 512],
                                wo_sb[:, h * DM + dmt * 128: h * DM + (dmt + 1) * 128],
                                ot_sb[:, h * LQ + n * 512: h * LQ + (n + 1) * 512],
                                start=(h == 0), stop=(h == G - 1),
                            )
                    o_out = outp.tile([128, LQ], f32)
                    nc.vector.tensor_copy(o_out[:, 0:LQ // 2], ps_out[:, 0:LQ // 2])
                    nc.scalar.activation(
                        o_out[:, LQ // 2:], ps_out[:, LQ // 2:],
                        mybir.ActivationFunctionType.Copy,
                    )
                    nc.sync.dma_start(outT[dmt * 128:(dmt + 1) * 128, :], o_out[:])
            wop.release()

    nc.finalize()
    return nc


def _get_nc():
    global _BUILT
    if _BUILT is None:
        _BUILT = _build()
    return _BUILT


def kernel(hidden_q, hidden_kv, attention_mask, position_bias, Wq, Wk, Wv, Wo,
           _trace=False):
    hidden_q = np.asarray(hidden_q, np.float32)
    hidden_kv = np.asarray(hidden_kv, np.float32)
    position_bias = np.asarray(position_bias, np.float32)
    Wq = np.asarray(Wq, np.float32)
    Wk = np.asarray(Wk, np.float32)
    Wv = np.asarray(Wv, np.float32)
    Wo = np.asarray(Wo, np.float32)
    # attention_mask is all-ones by problem spec; masking is a no-op.

    inv4 = 1.0 / np.sqrt(np.sqrt(np.float32(DH)))
    GW = G * DH

    def sb_layout(a, cast_bf16=True):
        # [dm, w] -> [128, KC*w] with contraction chunk kc at cols [kc*w,(kc+1)*w)
        w = a.shape[1]
        out = np.ascontiguousarray(
            a.reshape(KC, 128, w).transpose(1, 0, 2).reshape(128, KC * w)
        )
        return np.asarray(out.astype(ml_dtypes.bfloat16)) if cast_bf16 else out

    # per-kv weight shards
    wq_s, wk_s, wv_s, wo_s, biasT_s = [], [], [], [], []
    WqT = (Wq.T * inv4).astype(np.float32)   # [dm, H*dh]
    WkT = (Wk.T * inv4).astype(np.float32)   # [dm, KV*dh]
    WvT = Wv.T.astype(np.float32)            # [dm, KV*dh]
    for kv in range(KV):
        wq_s.append(sb_layout(np.ascontiguousarray(WqT[:, kv * GW:(kv + 1) * GW])))
        wk_s.append(sb_layout(np.ascontiguousarray(WkT[:, kv * DH:(kv + 1) * DH])))
        wv_s.append(sb_layout(np.ascontiguousarray(WvT[:, kv * DH:(kv + 1) * DH])))
        # wo layout: [128(dh), G*dm]; head h cols = Wo[:, kv*GW+h*DH : +DH].T
        wo_kv = Wo[:, kv * GW:(kv + 1) * GW].T  # [GW, dm]
        wo_s.append(np.asarray(np.ascontiguousarray(
            wo_kv.reshape(G, DH, DM).transpose(1, 0, 2).reshape(128, G * DM)
        ).astype(ml_dtypes.bfloat16)))
        biasT_s.append(np.ascontiguousarray(
            position_bias[kv * G:(kv + 1) * G].transpose(0, 2, 1)
        ).astype(ml_dtypes.bfloat16))

    hqT = [np.asarray(np.ascontiguousarray(hidden_q[b].T).astype(ml_dtypes.bfloat16))
           for b in range(B)]
    hkvT = [np.asarray(np.ascontiguousarray(hidden_kv[b].T).astype(ml_dtypes.bfloat16))
            for b in range(B)]
    ones_arr = np.ones((128, 1), np.float32)
    ident_arr = np.asarray(np.eye(128, dtype=np.float32).astype(ml_dtypes.bfloat16))
    ones_row_arr = np.ones((1, 128), np.float32)

    in_maps = []
    for core in range(8):
        b, kv = divmod(core, KV)
        in_maps.append({
            "hqT": hqT[b], "hkvT": hkvT[b],
            "wq": wq_s[kv], "wk": wk_s[kv], "wv": wv_s[kv], "wo": wo_s[kv],
            "biasT": np.asarray(biasT_s[kv]),
            "ones_in": ones_arr, "ones_row": ones_row_arr, "ident": ident_arr,
        })

    nc = _get_nc()
    res = run_bass_kernel_spmd(nc, in_maps, core_ids=list(range(8)), trace=_trace)
    kernel.last_exec_time_ns = res.exec_time_ns

    out = np.empty((B, LQ, DM), np.float32)
    for b in range(B):
        acc = res.results[b * KV]["outT"].astype(np.float64)
        for kv in range(1, KV):
            acc += res.results[b * KV + kv]["outT"]
        out[b] = acc.T.astype(np.float32)
    return out
